# revision 20
# baseline (speedup 1.0000x reference)
"""Trainium2 Bass kernel for nn_CommNetActor (gnn_message_passing).

Algebraic collapse: every comm layer is linear (no activation), so the whole
post-sigmoid network folds into per-agent decoder matrices on the host:

    out[b] = sum_a sigmoid(O[b,a] @ W_enc + b_enc) @ Z_a + r

Device pipeline (batch-sharded, 8192/8 = 1024 batches = 65536 tokens/core):
  - host casts O to bf16 (same numerics as the old on-device GPSIMD cast)
  - HWDGE xbar DMA-transpose loads O feature-major straight from HBM
    (no PE transposes, no GPSIMD cast, no PSUM->SBUF copies)
  - encoder: two col-group-tiled bf16 matmuls put agents a<32 on PSUM
    partitions 0-63 and a>=32 on 64-127
  - ACT sigmoid (+bias) -> bf16 arena [128, batch*pair] layout
  - decoder: 32 K=128 matmuls per group (two agents per matmul) accumulate
    over a PSUM strip [32, batches]
  - +r bias, PE transpose back to batch-major, single batched output store
"""

import sys
import numpy as np

sys.path.insert(0, "/opt/trn_rl_repo")

import ml_dtypes

BATCH, N_AGENTS, OBS_DIM, D, N_ACT = 8192, 64, 128, 64, 32
N_CORES = 8
B_CORE = BATCH // N_CORES              # 1024 batches per core
TOK_CORE = B_CORE * N_AGENTS           # 65536 tokens per core
NT = 1024                              # tokens per super-tile (16 batches)
N_ST = TOK_CORE // NT                  # 64 super-tiles
SG = 16                                # super-tiles per decoder group
N_G = N_ST // SG                       # 4 groups
GB = SG * (NT // N_AGENTS)             # 512 batches per group
DMA_ST = 8                             # super-tiles per input DMA (2 MB)

_CACHE = {}


def _fold_weights(W_enc, b_enc, Ws, bs, W_dec, b_dec):
    """Fold the 4 linear comm layers + decoder into Zdec [64,64,32] and r[32]."""
    A = N_AGENTS
    I = np.eye(D)
    Gamma = I.copy()
    E = np.zeros((D, D))
    c = np.zeros(D)
    Lam = I.copy()
    d = np.zeros(D)
    for W, b in zip(Ws, bs):
        W = W.astype(np.float64)
        b = b.astype(np.float64)
        Wt, Wb = W[:D], W[D:]
        V = Wb / A
        Wp = Wt - V
        U = Wt + (A - 1) * V
        E, c = E @ Wp + Lam @ V, c @ Wp + d @ V + b
        Gamma = Gamma @ Wp
        Lam, d = Lam @ U, d @ U + A * b
    Wd = W_dec.astype(np.float64).reshape(A, D, N_ACT)
    Wsum = Wd.sum(axis=0)
    Zdec = np.einsum("ij,ajk->aik", Gamma, Wd) + (E @ Wsum)[None]
    r = c @ Wsum + b_dec.astype(np.float64)
    return Zdec, r


def _build(loop_reps=1):
    import concourse.bass as bass
    import concourse.bacc as bacc
    import concourse.tile as tile
    from concourse import mybir
    from concourse._compat import get_trn_type

    f32 = mybir.dt.float32
    bf16 = mybir.dt.bfloat16

    nc = bacc.Bacc(get_trn_type() or "TRN2", target_bir_lowering=False,
                   debug=False, enable_asserts=True, num_devices=N_CORES)

    O_d = nc.dram_tensor("Obf", [TOK_CORE, OBS_DIM], bf16, kind="ExternalInput")
    wenc_d = nc.dram_tensor("Wenc", [OBS_DIM, D], bf16, kind="ExternalInput")
    benc_d = nc.dram_tensor("benc128", [128, 1], f32, kind="ExternalInput")
    zpair_d = nc.dram_tensor("Zpair", [128, 32, N_ACT], bf16,
                             kind="ExternalInput")
    r_d = nc.dram_tensor("r2", [32, 1], f32, kind="ExternalInput")
    idf_d = nc.dram_tensor("ident32", [32, 32], f32, kind="ExternalInput")
    out_d = nc.dram_tensor("out", [B_CORE, N_ACT], f32, kind="ExternalOutput")

    O_ap = O_d.ap()
    out_ap = out_d.ap()

    with tile.TileContext(nc) as tc:
        with (
            tc.tile_pool(name="const", bufs=1) as const_pool,
            tc.tile_pool(name="otsb", bufs=3) as ot_pool,
            tc.tile_pool(name="arena", bufs=2) as arena_pool,
            tc.tile_pool(name="outsb", bufs=2) as outsb_pool,
            tc.tile_pool(name="outt", bufs=2) as outt_pool,
            tc.tile_pool(name="ph", bufs=4, space="PSUM") as ph_pool,
            tc.tile_pool(name="pd", bufs=2, space="PSUM") as pd_pool,
            tc.tile_pool(name="po", bufs=1, space="PSUM") as po_pool,
        ):
            # constants
            wenc = const_pool.tile([OBS_DIM, D], bf16)
            nc.sync.dma_start(out=wenc[:], in_=wenc_d.ap())
            benc = const_pool.tile([128, 1], f32)
            nc.sync.dma_start(out=benc[:], in_=benc_d.ap())
            zpair = const_pool.tile([128, 32, N_ACT], bf16)
            nc.sync.dma_start(out=zpair[:], in_=zpair_d.ap())
            r2 = const_pool.tile([32, 1], f32)
            nc.sync.dma_start(out=r2[:], in_=r_d.ap())
            idf = const_pool.tile([32, 32], f32)
            nc.sync.dma_start(out=idf[:], in_=idf_d.ap())

            import contextlib
            loop_cm = (tc.For_i(0, loop_reps, 1) if loop_reps > 1
                       else contextlib.nullcontext())
            with loop_cm:
                nch = GB // 128        # output transpose chunks per group
                outt = outt_pool.tile([128, N_G * nch * N_ACT], f32)

                def dec_steps(g, arena):
                    """Decoder + output stage for group g as small steps,
                    interleaved between later encoder matmuls so the PE
                    FIFO never starves the encoder->ACT pipeline."""
                    pd = pd_pool.tile([32, GB], f32, name="pd")
                    for p in range(32):
                        nc.tensor.matmul(pd[:], zpair[:, p, :],
                                         arena[:, p * GB:(p + 1) * GB],
                                         start=(p == 0), stop=(p == 31))
                        if p % 2 == 1:
                            yield
                    sab = outsb_pool.tile([32, GB], f32, name="sab")
                    nc.scalar.add(sab[:], pd[:], add=r2[:])
                    yield
                    po = po_pool.tile([128, nch * N_ACT], f32, name="po")
                    for ch in range(nch):
                        nc.tensor.matmul(
                            po[:, ch * N_ACT:(ch + 1) * N_ACT],
                            sab[:, ch * 128:(ch + 1) * 128], idf[:],
                            start=True, stop=True)
                    nc.vector.tensor_copy(
                        outt[:, g * nch * N_ACT:(g + 1) * nch * N_ACT],
                        po[:])
                    yield

                pending = None
                for g in range(N_G):
                    arena = arena_pool.tile([128, SG * 512], bf16,
                                            name="arena")
                    ot = None
                    for sl in range(SG):
                        st = g * SG + sl
                        if st % DMA_ST == 0:
                            ot = ot_pool.tile([128, DMA_ST * NT], bf16,
                                              name="ot")
                            nc.sync.dma_start(
                                out=ot[:],
                                in_=O_ap[st * NT:(st + DMA_ST) * NT, :],
                                transpose=True)
                        sub = ot[:, (st % DMA_ST) * NT:(st % DMA_ST + 1) * NT]
                        # stream agent-major so arena lands pair-major and
                        # the decoder reads contiguous slabs
                        otr = sub.rearrange("p (b a) -> p a b", a=N_AGENTS)
                        ph = ph_pool.tile([128, 512], f32, name="ph")
                        nc.tensor.matmul(ph[0:64, :], wenc[:],
                                         otr[:, 0:32, :],
                                         start=True, stop=True,
                                         tile_position=(0, 0))
                        nc.tensor.matmul(ph[64:128, :], wenc[:],
                                         otr[:, 32:64, :],
                                         start=True, stop=True,
                                         tile_position=(0, 64))
                        # pair-major arena: cols = (pair a: 32, st: SG, b: 16)
                        a2 = arena[:].rearrange("q (a s2 b) -> q a s2 b",
                                                a=32, s2=SG)
                        nc.scalar.activation(
                            out=a2[:, :, sl, :],
                            in_=ph[:],
                            func=mybir.ActivationFunctionType.Sigmoid,
                            bias=benc[:])
                        if pending is not None:
                            next(pending, None)
                    if pending is not None:
                        for _ in pending:
                            pass
                    pending = dec_steps(g, arena)
                # flush the last group's decoder
                for _ in pending:
                    pass

                nc.sync.dma_start(
                    out=out_ap.rearrange("(g ch p) c -> p g ch c",
                                         g=N_G, ch=nch, p=128),
                    in_=outt[:].rearrange("p (g ch c) -> p g ch c",
                                          g=N_G, ch=nch))

    nc.compile()
    return nc


def _prep_inputs(inputs):
    W_enc = np.asarray(inputs["W_enc"], dtype=np.float32)
    b_enc = np.asarray(inputs["b_enc"], dtype=np.float32)
    Ws = [np.asarray(inputs[f"W{k}"], dtype=np.float32) for k in (1, 2, 3, 4)]
    bs = [np.asarray(inputs[f"b{k}"], dtype=np.float32) for k in (1, 2, 3, 4)]
    W_dec = np.asarray(inputs["W_dec"], dtype=np.float32)
    b_dec = np.asarray(inputs["b_dec"], dtype=np.float32)

    Zdec, r = _fold_weights(W_enc, b_enc, Ws, bs, W_dec, b_dec)
    zdev = np.ascontiguousarray(Zdec.transpose(1, 0, 2))  # [64 d, 64 a, 32]
    zpair = np.ascontiguousarray(np.concatenate(
        [zdev[:, 0:32, :], zdev[:, 32:64, :]], axis=0)).astype(
            ml_dtypes.bfloat16)                           # [128, 32, 32]
    benc128 = np.concatenate([b_enc, b_enc]).reshape(128, 1).astype(np.float32)
    r2 = r.reshape(32, 1).astype(np.float32)

    O = np.asarray(inputs["O"], dtype=np.float32)
    Obf = O.astype(ml_dtypes.bfloat16)
    common = {
        "Wenc": np.ascontiguousarray(W_enc).astype(ml_dtypes.bfloat16),
        "benc128": benc128,
        "Zpair": zpair,
        "r2": r2,
        "ident32": np.eye(32, dtype=np.float32),
    }
    in_maps = []
    for c in range(N_CORES):
        o_shard = np.ascontiguousarray(
            Obf[c * B_CORE:(c + 1) * B_CORE].reshape(TOK_CORE, OBS_DIM))
        in_maps.append({"Obf": o_shard, **common})
    return in_maps


def _run(inputs, trace=False):
    from concourse.bass_utils import run_bass_kernel_spmd

    if "nc" not in _CACHE:
        _CACHE["nc"] = _build()
    nc = _CACHE["nc"]
    in_maps = _prep_inputs(inputs)
    res = run_bass_kernel_spmd(nc, in_maps, core_ids=list(range(N_CORES)),
                               trace=trace)
    out = np.concatenate(
        [res.results[c]["out"] for c in range(N_CORES)], axis=0)
    return out.astype(np.float32), res


def kernel(**inputs):
    out, _ = _run(inputs, trace=False)
    return out


# revision 29
# speedup vs baseline: 1.1136x; 1.1136x over previous
"""Trainium2 Bass kernel for nn_CommNetActor (gnn_message_passing).

Algebraic collapse: every comm layer is linear (no activation), so the whole
post-sigmoid network folds into per-agent decoder matrices on the host:

    out[b] = sum_a sigmoid(O[b,a] @ W_enc + b_enc) @ Z_a + r

Device pipeline (batch-sharded, 8192/8 = 1024 batches = 65536 tokens/core):
  - host casts O to bf16 (same numerics as the old on-device GPSIMD cast)
  - HWDGE xbar DMA-transpose loads O feature-major straight from HBM
    (no PE transposes, no GPSIMD cast, no PSUM->SBUF copies)
  - encoder: two col-group-tiled bf16 matmuls put agents a<32 on PSUM
    partitions 0-63 and a>=32 on 64-127
  - ACT sigmoid (+bias) -> bf16 arena [128, batch*pair] layout
  - decoder: 32 K=128 matmuls per group (two agents per matmul) accumulate
    over a PSUM strip [32, batches]
  - +r bias, PE transpose back to batch-major, single batched output store
"""

import sys
import numpy as np

sys.path.insert(0, "/opt/trn_rl_repo")

import ml_dtypes

BATCH, N_AGENTS, OBS_DIM, D, N_ACT = 8192, 64, 128, 64, 32
N_CORES = 8
B_CORE = BATCH // N_CORES              # 1024 batches per core
TOK_CORE = B_CORE * N_AGENTS           # 65536 tokens per core
NT = 1024                              # tokens per super-tile (16 batches)
N_ST = TOK_CORE // NT                  # 64 super-tiles
SG = 32                                # super-tiles per decoder group
N_G = N_ST // SG                       # 2 groups
GB = SG * (NT // N_AGENTS)             # 512 batches per group
DMA_ST = 8                             # super-tiles per input DMA (2 MB)

_CACHE = {}


def _fold_weights(W_enc, b_enc, Ws, bs, W_dec, b_dec):
    """Fold the 4 linear comm layers + decoder into Zdec [64,64,32] and r[32]."""
    A = N_AGENTS
    I = np.eye(D)
    Gamma = I.copy()
    E = np.zeros((D, D))
    c = np.zeros(D)
    Lam = I.copy()
    d = np.zeros(D)
    for W, b in zip(Ws, bs):
        W = W.astype(np.float64)
        b = b.astype(np.float64)
        Wt, Wb = W[:D], W[D:]
        V = Wb / A
        Wp = Wt - V
        U = Wt + (A - 1) * V
        E, c = E @ Wp + Lam @ V, c @ Wp + d @ V + b
        Gamma = Gamma @ Wp
        Lam, d = Lam @ U, d @ U + A * b
    Wd = W_dec.astype(np.float64).reshape(A, D, N_ACT)
    Wsum = Wd.sum(axis=0)
    Zdec = np.einsum("ij,ajk->aik", Gamma, Wd) + (E @ Wsum)[None]
    r = c @ Wsum + b_dec.astype(np.float64)
    return Zdec, r


def _build(loop_reps=1, skip_dma=False, skip_compute=False, skip_dec=False):
    import concourse.bass as bass
    import concourse.bacc as bacc
    import concourse.tile as tile
    from concourse import mybir
    from concourse._compat import get_trn_type

    f32 = mybir.dt.float32
    bf16 = mybir.dt.bfloat16

    nc = bacc.Bacc(get_trn_type() or "TRN2", target_bir_lowering=False,
                   debug=False, enable_asserts=True, num_devices=N_CORES)

    O_d = nc.dram_tensor("Obf", [TOK_CORE, OBS_DIM], bf16, kind="ExternalInput")
    wenc_d = nc.dram_tensor("Wenc", [OBS_DIM, D], bf16, kind="ExternalInput")
    benc_d = nc.dram_tensor("benc128", [128, 1], f32, kind="ExternalInput")
    zpair_d = nc.dram_tensor("Zpair", [128, 32, N_ACT], bf16,
                             kind="ExternalInput")
    r_d = nc.dram_tensor("r2", [32, 1], f32, kind="ExternalInput")
    idf_d = nc.dram_tensor("ident32", [32, 32], f32, kind="ExternalInput")
    out_d = nc.dram_tensor("out", [B_CORE, N_ACT], f32, kind="ExternalOutput")

    O_ap = O_d.ap()
    out_ap = out_d.ap()

    with tile.TileContext(nc) as tc:
        with (
            tc.tile_pool(name="const", bufs=1) as const_pool,
            tc.tile_pool(name="otsb", bufs=3) as ot_pool,
            tc.tile_pool(name="arena", bufs=2) as arena_pool,
            tc.tile_pool(name="outsb", bufs=2) as outsb_pool,
            tc.tile_pool(name="outt", bufs=2) as outt_pool,
            tc.tile_pool(name="ph", bufs=4, space="PSUM") as ph_pool,
            tc.tile_pool(name="pd", bufs=2, space="PSUM") as pd_pool,
            tc.tile_pool(name="po", bufs=1, space="PSUM") as po_pool,
        ):
            # constants
            wenc = const_pool.tile([OBS_DIM, D], bf16)
            nc.sync.dma_start(out=wenc[:], in_=wenc_d.ap())
            benc = const_pool.tile([128, 1], f32)
            nc.sync.dma_start(out=benc[:], in_=benc_d.ap())
            zpair = const_pool.tile([128, 32, N_ACT], bf16)
            nc.sync.dma_start(out=zpair[:], in_=zpair_d.ap())
            r2 = const_pool.tile([32, 1], f32)
            nc.sync.dma_start(out=r2[:], in_=r_d.ap())
            idf = const_pool.tile([32, 32], f32)
            nc.sync.dma_start(out=idf[:], in_=idf_d.ap())

            otc = None
            if skip_dma:
                otc = const_pool.tile([128, DMA_ST * NT], bf16, name="otc")
                nc.sync.dma_start(out=otc[:], in_=O_ap[0:DMA_ST * NT, :],
                                  transpose=True)

            import contextlib
            loop_cm = (tc.For_i(0, loop_reps, 1) if loop_reps > 1
                       else contextlib.nullcontext())
            with loop_cm:
                nch = GB // 128        # output transpose chunks per group
                do_out = not (skip_compute or skip_dec)
                outt = (outt_pool.tile([128, N_G * nch * N_ACT], f32,
                                       name="outt") if do_out else None)

                def dec_steps(g, arena):
                    """Decoder + output stage for group g as small steps,
                    interleaved between later encoder matmuls so the PE
                    FIFO never starves the encoder->ACT pipeline."""
                    pd = pd_pool.tile([32, GB], f32, name="pd")
                    for p in range(32):
                        nc.tensor.matmul(pd[:], zpair[:, p, :],
                                         arena[:, p * GB:(p + 1) * GB],
                                         start=(p == 0), stop=(p == 31))
                        if p % 2 == 1:
                            yield
                    sab = outsb_pool.tile([32, GB], f32, name="sab")
                    nc.scalar.add(sab[:], pd[:], add=r2[:])
                    yield
                    po = po_pool.tile([128, nch * N_ACT], f32, name="po")
                    for ch in range(nch):
                        nc.tensor.matmul(
                            po[:, ch * N_ACT:(ch + 1) * N_ACT],
                            sab[:, ch * 128:(ch + 1) * 128], idf[:],
                            start=True, stop=True)
                    nc.vector.tensor_copy(
                        outt[:, g * nch * N_ACT:(g + 1) * nch * N_ACT],
                        po[:])
                    yield

                pending = None
                for g in range(N_G):
                    arena = (arena_pool.tile([128, SG * 512], bf16,
                                             name="arena")
                             if not skip_compute else None)
                    ot = None
                    for sl in range(SG):
                        st = g * SG + sl
                        if st % DMA_ST == 0 and not skip_dma:
                            ot = ot_pool.tile([128, DMA_ST * NT], bf16,
                                              name="ot")
                            nc.sync.dma_start(
                                out=ot[:],
                                in_=O_ap[st * NT:(st + DMA_ST) * NT, :],
                                transpose=True)
                        if skip_dma:
                            ot = otc
                        if skip_compute:
                            continue
                        sub = ot[:, (st % DMA_ST) * NT:(st % DMA_ST + 1) * NT]
                        # stream agent-major so arena lands pair-major and
                        # the decoder reads contiguous slabs
                        otr = sub.rearrange("p (b a) -> p a b", a=N_AGENTS)
                        ph = ph_pool.tile([128, 512], f32, name="ph")
                        nc.tensor.matmul(ph[0:64, :], wenc[:],
                                         otr[:, 0:32, :],
                                         start=True, stop=True,
                                         tile_position=(0, 0))
                        nc.tensor.matmul(ph[64:128, :], wenc[:],
                                         otr[:, 32:64, :],
                                         start=True, stop=True,
                                         tile_position=(0, 64))
                        # pair-major arena: cols = (pair a: 32, st: SG, b: 16)
                        a2 = arena[:].rearrange("q (a s2 b) -> q a s2 b",
                                                a=32, s2=SG)
                        nc.scalar.activation(
                            out=a2[:, :, sl, :],
                            in_=ph[:],
                            func=mybir.ActivationFunctionType.Sigmoid,
                            bias=benc[:])
                    # run the decoder right after the group's last ST
                    if not (skip_compute or skip_dec):
                        pending = dec_steps(g, arena)
                        for _ in pending:
                            pass

                if do_out:
                    nc.sync.dma_start(
                        out=out_ap.rearrange("(g ch p) c -> p g ch c",
                                             g=N_G, ch=nch, p=128),
                        in_=outt[:].rearrange("p (g ch c) -> p g ch c",
                                              g=N_G, ch=nch))

    nc.compile()
    return nc


def _prep_inputs(inputs):
    W_enc = np.asarray(inputs["W_enc"], dtype=np.float32)
    b_enc = np.asarray(inputs["b_enc"], dtype=np.float32)
    Ws = [np.asarray(inputs[f"W{k}"], dtype=np.float32) for k in (1, 2, 3, 4)]
    bs = [np.asarray(inputs[f"b{k}"], dtype=np.float32) for k in (1, 2, 3, 4)]
    W_dec = np.asarray(inputs["W_dec"], dtype=np.float32)
    b_dec = np.asarray(inputs["b_dec"], dtype=np.float32)

    Zdec, r = _fold_weights(W_enc, b_enc, Ws, bs, W_dec, b_dec)
    zdev = np.ascontiguousarray(Zdec.transpose(1, 0, 2))  # [64 d, 64 a, 32]
    zpair = np.ascontiguousarray(np.concatenate(
        [zdev[:, 0:32, :], zdev[:, 32:64, :]], axis=0)).astype(
            ml_dtypes.bfloat16)                           # [128, 32, 32]
    benc128 = np.concatenate([b_enc, b_enc]).reshape(128, 1).astype(np.float32)
    r2 = r.reshape(32, 1).astype(np.float32)

    O = np.asarray(inputs["O"], dtype=np.float32)
    Obf = O.astype(ml_dtypes.bfloat16)
    common = {
        "Wenc": np.ascontiguousarray(W_enc).astype(ml_dtypes.bfloat16),
        "benc128": benc128,
        "Zpair": zpair,
        "r2": r2,
        "ident32": np.eye(32, dtype=np.float32),
    }
    in_maps = []
    for c in range(N_CORES):
        o_shard = np.ascontiguousarray(
            Obf[c * B_CORE:(c + 1) * B_CORE].reshape(TOK_CORE, OBS_DIM))
        in_maps.append({"Obf": o_shard, **common})
    return in_maps


def _run(inputs, trace=False):
    from concourse.bass_utils import run_bass_kernel_spmd

    if "nc" not in _CACHE:
        _CACHE["nc"] = _build()
    nc = _CACHE["nc"]
    in_maps = _prep_inputs(inputs)
    res = run_bass_kernel_spmd(nc, in_maps, core_ids=list(range(N_CORES)),
                               trace=trace)
    out = np.concatenate(
        [res.results[c]["out"] for c in range(N_CORES)], axis=0)
    return out.astype(np.float32), res


def kernel(**inputs):
    out, _ = _run(inputs, trace=False)
    return out
